# revision 12
# baseline (speedup 1.0000x reference)
"""AttentionLayerWithRPR on 8 trn2 NeuronCores.

Sharding: (batch, sq-half) -> 8 cores. Core (b, s) computes batch b, all 8
heads, query rows [s*512, (s+1)*512).

Per-core pipeline (normal layout, scores [q=partitions, k=free]):
  - load q/k/v natural, PE-transpose 128x128 blocks -> qT/kT/vT
  - projections on PE: qhT/khT = W.T @ xT (f32), vh natural (bf16)
  - QR[h] = qh . krpr^T  ([q, 11] per head) on PE
  - masks m_r = (rpr == r) as bf16, shared across heads
  - scores = qhT.T @ khT (PSUM), bias added via 11 scalar_tensor_tensor
    passes (mask_r * QR[:,r] + acc); buckets are disjoint so the bf16
    chain rounds each element exactly once
  - E = exp(S/8) on ACT; bucket sums P[q,r] via STT accum_out
  - PV: PE-transpose E tiles, ctx = E^T.T @ vh + P^T.T @ krpr (one PSUM
    accumulation group); denominator = sum_r P; out = ctx * recip + bv
"""

import os
from contextlib import ExitStack

import numpy as np

import concourse.bass as bass
import concourse.bacc as bacc
import concourse.mybir as mybir
from concourse.tile import TileContext
from concourse.masks import make_identity

B, S, H, DH = 4, 1024, 8, 64
D = H * DH  # 512
NR = 11
SQ = S // 2  # per-core query rows
NCORES = 8

F32 = mybir.dt.float32
BF16 = mybir.dt.bfloat16
I32 = mybir.dt.int32
OP = mybir.AluOpType
AF = mybir.ActivationFunctionType
AX = mybir.AxisListType

NT = D // 128   # 4 d-in / d-out tiles
QT = SQ // 128  # 4 q tiles
KT = S // 128   # 8 k tiles


def _build():
    nc = bacc.Bacc()
    q_d = nc.dram_tensor("q", [SQ, D], F32, kind="ExternalInput")
    k_d = nc.dram_tensor("k", [S, D], F32, kind="ExternalInput")
    v_d = nc.dram_tensor("v", [S, D], F32, kind="ExternalInput")
    rpr_d = nc.dram_tensor("rpr", [SQ, S], I32, kind="ExternalInput")
    wq_d = nc.dram_tensor("wq", [D, D], F32, kind="ExternalInput")
    wk_d = nc.dram_tensor("wk", [D, D], F32, kind="ExternalInput")
    wv_d = nc.dram_tensor("wv", [D, D], F32, kind="ExternalInput")
    bq_d = nc.dram_tensor("bq", [D], F32, kind="ExternalInput")
    bk_d = nc.dram_tensor("bk", [D], F32, kind="ExternalInput")
    bv_d = nc.dram_tensor("bv", [D], F32, kind="ExternalInput")
    krpr_d = nc.dram_tensor("krpr", [NR, DH], F32, kind="ExternalInput")
    out_d = nc.dram_tensor("out", [SQ, D], F32, kind="ExternalOutput")

    with TileContext(nc) as tc, ExitStack() as ctx:
        const = ctx.enter_context(tc.tile_pool(name="const", bufs=1))

        id_f32 = const.tile([128, 128], F32, tag="id_f32", name="id_f32")
        make_identity(nc, id_f32)
        id_bf = const.tile([128, 128], BF16, tag="id_bf", name="id_bf")
        make_identity(nc, id_bf)

        # --- weights / small constants -------------------------------------
        wq_sb = [const.tile([128, D], F32, tag=f"wq{i}", name=f"wq{i}") for i in range(NT)]
        wk_sb = [const.tile([128, D], F32, tag=f"wk{i}", name=f"wk{i}") for i in range(NT)]
        wv_sb = [const.tile([128, D], F32, tag=f"wv{i}", name=f"wv{i}") for i in range(NT)]
        for i in range(NT):
            nc.sync.dma_start(out=wq_sb[i], in_=wq_d[i * 128:(i + 1) * 128, :])
            nc.sync.dma_start(out=wk_sb[i], in_=wk_d[i * 128:(i + 1) * 128, :])
            nc.sync.dma_start(out=wv_sb[i], in_=wv_d[i * 128:(i + 1) * 128, :])
        bq_sb = [const.tile([128, 1], F32, tag=f"bq{i}", name=f"bq{i}") for i in range(NT)]
        bk_sb = [const.tile([128, 1], F32, tag=f"bk{i}", name=f"bk{i}") for i in range(NT)]
        for i in range(NT):
            nc.sync.dma_start(
                out=bq_sb[i],
                in_=bq_d[i * 128:(i + 1) * 128].rearrange("(p o) -> p o", o=1))
            nc.sync.dma_start(
                out=bk_sb[i],
                in_=bk_d[i * 128:(i + 1) * 128].rearrange("(p o) -> p o", o=1))
        krpr_sb = const.tile([NR, DH], F32, tag="krpr", name="krpr")
        nc.sync.dma_start(out=krpr_sb, in_=krpr_d[:, :])
        bv_row0 = const.tile([1, D], F32, tag="bv_row0", name="bv_row0")
        nc.sync.dma_start(out=bv_row0, in_=bv_d.rearrange("(o d) -> o d", o=1))
        bv_row = const.tile([1, D], F32, tag="bv_row", name="bv_row")
        nc.vector.tensor_copy(bv_row, bv_row0)
        ones_col = const.tile([1, 128], F32, tag="ones_col", name="ones_col")
        nc.vector.memset(ones_col, 1.0)

        # bv broadcast to all partitions via a K=1 matmul (both matmul
        # operands are DVE-produced so the fused LDW carries one wait)
        bv_full = const.tile([128, D], F32, tag="bv_full", name="bv_full")
        with tc.tile_pool(name="bvps", bufs=1, space="PSUM") as bvps:
            bvp = bvps.tile([128, D], F32)
            nc.tensor.matmul(bvp[:, 0:D], ones_col, bv_row, start=True, stop=True)
            nc.scalar.copy(bv_full, bvp)

        # --- persistent activations ----------------------------------------
        qhT = [const.tile([128, SQ], F32, tag=f"qhT{i}", name=f"qhT{i}") for i in range(NT)]
        khT = [const.tile([128, S], F32, tag=f"khT{i}", name=f"khT{i}") for i in range(NT)]
        vh = [const.tile([128, D], BF16, tag=f"vh{i}", name=f"vh{i}") for i in range(KT)]
        QR = const.tile([128, QT * H * NR], F32, tag="QR", name="QR")

        # --- stage A/B: transpose inputs + projections ----------------------
        with tc.tile_pool(name="ldnat", bufs=3) as ldnat, \
             tc.tile_pool(name="xT", bufs=1) as xTp, \
             tc.tile_pool(name="tps", bufs=2, space="PSUM") as tps, \
             tc.tile_pool(name="pps", bufs=2, space="PSUM") as pps:

            qT = [xTp.tile([128, SQ], F32, tag=f"qT{i}", name=f"qT{i}") for i in range(NT)]
            kT = [xTp.tile([128, S], F32, tag=f"kT{i}", name=f"kT{i}") for i in range(NT)]
            vT = [xTp.tile([128, S], F32, tag=f"vT{i}", name=f"vT{i}") for i in range(NT)]

            def load_transposed(dram, nrows, dst):
                for rt in range(nrows // 128):
                    nat = ldnat.tile([128, D], F32, tag="nat", name="nat")
                    nc.sync.dma_start(
                        out=nat, in_=dram[rt * 128:(rt + 1) * 128, :])
                    for dt in range(NT):
                        tp = tps.tile([128, 128], F32, tag="tp", name="tp")
                        nc.tensor.transpose(
                            tp, nat[:, dt * 128:(dt + 1) * 128], id_f32)
                        if dt % 2:
                            nc.scalar.copy(
                                dst[dt][:, rt * 128:(rt + 1) * 128], tp)
                        else:
                            nc.vector.tensor_copy(
                                dst[dt][:, rt * 128:(rt + 1) * 128], tp)

            load_transposed(q_d, SQ, qT)
            load_transposed(k_d, S, kT)
            load_transposed(v_d, S, vT)

            # qhT[t][dout_local, row] = sum_di wq[di, t*128+dout].T qT
            for t in range(NT):
                ps = pps.tile([128, SQ], F32, tag="pp", name="pp")
                for half in range(SQ // 512):
                    sl = slice(half * 512, (half + 1) * 512)
                    for di in range(NT):
                        nc.tensor.matmul(
                            ps[:, sl], wq_sb[di][:, t * 128:(t + 1) * 128],
                            qT[di][:, sl], start=(di == 0), stop=(di == NT - 1))
                nc.scalar.activation(qhT[t], ps, AF.Identity, bias=bq_sb[t])
            for t in range(NT):
                for half in range(S // 512):
                    sl = slice(half * 512, (half + 1) * 512)
                    ps = pps.tile([128, 512], F32, tag="pp", name="ppk")
                    for di in range(NT):
                        nc.tensor.matmul(
                            ps, wk_sb[di][:, t * 128:(t + 1) * 128],
                            kT[di][:, sl], start=(di == 0), stop=(di == NT - 1))
                    nc.scalar.activation(
                        khT[t][:, sl], ps, AF.Identity, bias=bk_sb[t])
            # vh natural (bf16, no bias: bv folded into the epilogue)
            for kt in range(KT):
                ps = pps.tile([128, D], F32, tag="pp", name="pp")
                for di in range(NT):
                    nc.tensor.matmul(
                        ps, vT[di][:, kt * 128:(kt + 1) * 128], wv_sb[di],
                        start=(di == 0), stop=(di == NT - 1))
                nc.vector.tensor_copy(vh[kt], ps)

            # krpr^T [64, 11], replicated in both partition halves so that
            # odd heads (qhT at partitions 64:128) see a matching base
            krprT = const.tile([128, NR], F32, tag="krprT", name="krprT")
            tpk = tps.tile([128, 128], F32, tag="tp", name="tp")
            nc.tensor.transpose(
                tpk[0:DH, 0:NR], krpr_sb, id_f32[0:NR, 0:NR])
            nc.vector.tensor_copy(krprT[0:DH, :], tpk[0:DH, 0:NR])
            nc.sync.dma_start(out=krprT[DH:128, :], in_=krprT[0:DH, :])

            # QR[:, (qt*H + h)*NR + r] = qh[h] . krpr[r]
            with tc.tile_pool(name="qrps", bufs=2, space="PSUM") as qrps:
                for qt in range(QT):
                    for h in range(H):
                        po = (h % 2) * 64
                        lh = qhT[h // 2][po:po + 64,
                                         qt * 128:(qt + 1) * 128]
                        ps = qrps.tile([128, NR], F32, tag="qr", name="qr")
                        nc.tensor.matmul(
                            ps, lh, krprT[po:po + DH, :], start=True, stop=True)
                        base = (qt * H + h) * NR
                        nc.vector.tensor_copy(QR[:, base:base + NR], ps)

        # --- stage C: attention ---------------------------------------------
        with tc.tile_pool(name="rpr", bufs=2) as rprp, \
             tc.tile_pool(name="masks", bufs=2) as maskp, \
             tc.tile_pool(name="sacc", bufs=4) as saccp, \
             tc.tile_pool(name="ep", bufs=3) as ep, \
             tc.tile_pool(name="etp", bufs=3) as etp, \
             tc.tile_pool(name="small", bufs=4) as smallp, \
             tc.tile_pool(name="outp", bufs=2) as outp, \
             tc.tile_pool(name="sps", bufs=2, space="PSUM") as sps, \
             tc.tile_pool(name="cps", bufs=1, space="PSUM") as cps, \
             tc.tile_pool(name="tps2", bufs=2, space="PSUM") as tps2:

            trash = const.tile([128, S], BF16, tag="trash", name="trash")

            for qt in range(QT):
                rpr_i = rprp.tile([128, S], I32, tag="rpri", name="rpri")
                nc.sync.dma_start(
                    out=rpr_i, in_=rpr_d[qt * 128:(qt + 1) * 128, :])
                rpr_bf = rprp.tile([128, S], BF16, tag="rprbf", name="rprbf")
                nc.vector.tensor_copy(rpr_bf, rpr_i)
                masks = []
                for r in range(NR):
                    m = maskp.tile([128, S], BF16, tag=f"mask{r}", name=f"mask{r}")
                    nc.vector.tensor_scalar(
                        out=m, in0=rpr_bf, scalar1=float(r), scalar2=None,
                        op0=OP.is_equal)
                    masks.append(m)

                out_sb = outp.tile([128, D], F32, tag="out", name="out")

                for h in range(H):
                    t, po = h // 2, (h % 2) * 64
                    qh_sl = qhT[t][po:po + 64, qt * 128:(qt + 1) * 128]
                    # scores
                    scp = sps.tile([128, S], F32, tag="sc", name="sc")
                    for half in range(2):
                        nc.tensor.matmul(
                            scp[:, half * 512:(half + 1) * 512], qh_sl,
                            khT[t][po:po + 64, half * 512:(half + 1) * 512],
                            start=True, stop=True)
                    # bias: S = scores + sum_r mask_r * QR[:, r]
                    qrb = (qt * H + h) * NR
                    s_prev = saccp.tile([128, S], BF16, tag="sa", name="sa")
                    nc.vector.scalar_tensor_tensor(
                        out=s_prev, in0=masks[0], scalar=QR[:, qrb:qrb + 1],
                        in1=scp, op0=OP.mult, op1=OP.add)
                    for r in range(1, NR):
                        s_new = saccp.tile([128, S], BF16, tag="sa", name="sa")
                        nc.vector.scalar_tensor_tensor(
                            out=s_new, in0=masks[r],
                            scalar=QR[:, qrb + r:qrb + r + 1],
                            in1=s_prev, op0=OP.mult, op1=OP.add)
                        s_prev = s_new
                    # E = exp(S/8)
                    e = ep.tile([128, S], BF16, tag="e", name="e")
                    nc.scalar.activation(e, s_prev, AF.Exp, scale=0.125)
                    # bucket sums P[:, r] = sum_k E*mask_r
                    P = smallp.tile([128, NR], F32, tag="P", name="P")
                    for r in range(NR):
                        nc.vector.scalar_tensor_tensor(
                            out=trash, in0=masks[r], scalar=1.0, in1=e,
                            op0=OP.mult, op1=OP.mult,
                            accum_out=P[:, r:r + 1])
                    den = smallp.tile([128, 1], F32, tag="den", name="den")
                    nc.vector.tensor_reduce(den, P, AX.X, OP.add)
                    rden = smallp.tile([128, 1], F32, tag="rden", name="rden")
                    nc.vector.reciprocal(rden, den)

                    # ctx = E^T.T @ vh + P^T.T @ krpr  (one PSUM group)
                    cxp = cps.tile([128, 64], F32, tag="cx", name="cx")
                    for kt in range(KT):
                        tp = tps2.tile([128, 128], BF16, tag="tpe", name="tpe")
                        nc.tensor.transpose(
                            tp, e[:, kt * 128:(kt + 1) * 128], id_bf)
                        et = etp.tile([128, 128], BF16, tag="et", name="et")
                        if kt % 2:
                            nc.scalar.copy(et, tp)
                        else:
                            nc.vector.tensor_copy(et, tp)
                        nc.tensor.matmul(
                            cxp, et, vh[kt][:, h * 64:(h + 1) * 64],
                            start=(kt == 0), stop=False)
                    # P^T via PE transpose, then contract r
                    ptp = tps2.tile([128, 128], F32, tag="ptp", name="ptp", bufs=1)
                    nc.tensor.transpose(ptp[0:NR, :], P, id_f32)
                    pts = smallp.tile([NR, 128], F32, tag="pts", name="pts")
                    nc.vector.tensor_copy(pts, ptp[0:NR, :])
                    nc.tensor.matmul(
                        cxp, pts, krpr_sb, start=False, stop=True)

                    # out = ctx * rden + bv
                    nc.vector.scalar_tensor_tensor(
                        out=out_sb[:, h * 64:(h + 1) * 64], in0=cxp,
                        scalar=rden, in1=bv_full[:, h * 64:(h + 1) * 64],
                        op0=OP.mult, op1=OP.add)

                nc.sync.dma_start(
                    out=out_d[qt * 128:(qt + 1) * 128, :], in_=out_sb)

    nc.finalize()
    return nc


_NC = None


def _get_nc():
    global _NC
    if _NC is None:
        _NC = _build()
    return _NC


def _run(inputs, trace=False):
    from concourse.bass_utils import run_bass_kernel_spmd

    q = np.asarray(inputs["q"], dtype=np.float32)
    k = np.asarray(inputs["k"], dtype=np.float32)
    v = np.asarray(inputs["v"], dtype=np.float32)
    rpr = np.asarray(inputs["rpr_matrix"], dtype=np.int32)
    krpr = np.asarray(inputs["krpr"], dtype=np.float32)

    in_maps = []
    for b in range(B):
        for s in range(S // SQ):
            in_maps.append({
                "q": np.ascontiguousarray(q[b, s * SQ:(s + 1) * SQ, :]),
                "k": np.ascontiguousarray(k[b]),
                "v": np.ascontiguousarray(v[b]),
                "rpr": np.ascontiguousarray(rpr[s * SQ:(s + 1) * SQ, :]),
                "wq": np.ascontiguousarray(inputs["wq_kernel"], dtype=np.float32),
                "wk": np.ascontiguousarray(inputs["wk_kernel"], dtype=np.float32),
                "wv": np.ascontiguousarray(inputs["wv_kernel"], dtype=np.float32),
                "bq": np.ascontiguousarray(inputs["wq_bias"], dtype=np.float32),
                "bk": np.ascontiguousarray(inputs["wk_bias"], dtype=np.float32),
                "bv": np.ascontiguousarray(inputs["wv_bias"], dtype=np.float32),
                "krpr": np.ascontiguousarray(krpr),
            })

    res = run_bass_kernel_spmd(
        _get_nc(), in_maps, core_ids=list(range(NCORES)), trace=trace)
    out = np.empty((B, S, D), dtype=np.float32)
    i = 0
    for b in range(B):
        for s in range(S // SQ):
            out[b, s * SQ:(s + 1) * SQ, :] = res.results[i]["out"]
            i += 1
    return out, res


def kernel(**inputs) -> np.ndarray:
    out, _ = _run(inputs, trace=False)
    return out


def kernel_traced(**inputs):
    out, res = _run(inputs, trace=True)
    return out, res
